# revision 9
# baseline (speedup 1.0000x reference)
"""Trainium2 Bass kernel for nn_AttentionKernel_Position_47502338294174.

Reference computation (B=32, D=H=512, S=4096):
    yh = y_history.transpose(0, 2, 1)                 # [B,S,D]
    k  = yh @ Wk_w.T + Wk_b + yh + pe                 # [B,S,H]
    q  = k[:, -1, :]
    out = softmax((k @ q) / sqrt(H))                  # [B,S]

Key algebraic reduction (K is never materialized):
    W' = Wk_w + I                  (folds the "+ yh" term; H == D)
    pb = pe.T + Wk_b[:, None]      # [H, S] host constant
    q[h]      = sum_d W'[h,d] y[d,S-1] + pb[h,S-1]
    v         = W'.T @ q
    scores[s] = v . y[:, s] + q . pb[:, s]
    out       = softmax(scores / sqrt(H))

This turns a 68-GFLOP batched matmul into a matvec streamed over y_history
(268 MB) -> the kernel is HBM-bound at ~93us/core across 8 cores.

Sharding: pure data parallel, 4 batch elements per core; W'/pb replicated.
"""

import math

import numpy as np

B, D, S, H = 32, 512, 4096, 512
NCORES = 8
BPC = B // NCORES  # batches per core
INV_SQRT_H = 1.0 / math.sqrt(H)
DC = D // 128  # 4 contraction chunks
ST = S // 512  # 8 score tiles

# test.py can flip these before calling kernel()
TRACE = False
LAST_RESULT = None
REPEAT = 1  # perf harness: repeat the whole per-core workload in one NEFF

_CACHED = None  # (nc_program, ) built once per process


def _sinusoidal_pe(seq_len, d_model):
    pos = np.arange(seq_len, dtype=np.float32)[:, None]
    div = np.exp(
        np.arange(0, d_model, 2, dtype=np.float32) * (-math.log(10000.0) / d_model)
    ).astype(np.float32)
    pe = np.zeros((seq_len, d_model), dtype=np.float32)
    pe[:, 0::2] = np.sin(pos * div)
    pe[:, 1::2] = np.cos(pos * div)
    return pe


def _split_sync_waits(nc, mybir, max_waits=1):
    """The walrus build in this env rejects instructions carrying more than
    one sync-wait command. Hoist excess waits onto preceding same-engine NoOp
    carriers (sequential waits AND together -> identical semantics)."""
    n = 0
    for f in nc.m.functions:
        for blk in f.blocks:
            out = []
            for inst in blk.instructions:
                si = getattr(inst, "sync_info", None)
                if si is not None and si.on_wait and len(si.on_wait) > max_waits:
                    waits = list(si.on_wait)
                    while len(waits) > max_waits:
                        chunk, waits = waits[:max_waits], waits[max_waits:]
                        out.append(
                            mybir.InstNoOp(
                                name=f"{inst.name}-wsplit{n}",
                                engine=inst.engine,
                                ins=[],
                                outs=[],
                                sync_info=mybir.SyncInfo(
                                    on_wait=chunk, on_update=[]
                                ),
                            )
                        )
                        n += 1
                    si.on_wait = waits
                out.append(inst)
            blk.instructions = out
    return n


def _build_program():
    import concourse.bass as bass  # noqa: F401
    import concourse.mybir as mybir
    import concourse.tile as tile
    fp32 = mybir.dt.float32
    nc = bass.Bass(
        "TRN2",
        target_bir_lowering=False,
        debug=False,
        enable_asserts=False,
        num_devices=1,
    )

    y = nc.dram_tensor("y", (BPC, D, S), fp32, kind="ExternalInput").ap()
    wp = nc.dram_tensor("wp", (H, D), fp32, kind="ExternalInput").ap()
    wpt = nc.dram_tensor("wpt", (D, H), fp32, kind="ExternalInput").ap()
    pb = nc.dram_tensor("pb", (H, S), fp32, kind="ExternalInput").ap()
    out = nc.dram_tensor("out", (BPC, S), fp32, kind="ExternalOutput").ap()

    with tile.TileContext(nc) as tc:
        with (
            tc.tile_pool(name="singles", bufs=1) as singles,
            tc.tile_pool(name="ypool", bufs=8) as ypool,
            tc.tile_pool(name="small", bufs=2) as small,
            tc.tile_pool(name="rows", bufs=2) as rows,
            tc.tile_pool(name="ps_small", bufs=2, space="PSUM") as ps_small,
            tc.tile_pool(name="ps_s", bufs=3, space="PSUM") as ps_s,
        ):
            # ---- replicated constants (loaded once) ----
            # wp_sb[p, hc*512 + d] = W'[hc*128 + p, d]
            wp_sb = singles.tile([128, DC, D], fp32)
            nc.sync.dma_start(out=wp_sb, in_=wp.rearrange("(hc p) d -> p hc d", p=128))
            # wpt_sb[p, dc, h] = W'.T[dc*128 + p, h]
            wpt_sb = singles.tile([128, DC, H], fp32)
            nc.sync.dma_start(
                out=wpt_sb, in_=wpt.rearrange("(dc p) h -> p dc h", p=128)
            )
            # pb_sb[p, hc, s] = pb[hc*128 + p, s]
            pb_sb = singles.tile([128, DC, S], fp32)
            nc.sync.dma_start(out=pb_sb, in_=pb.rearrange("(hc p) s -> p hc s", p=128))

            for b in [bb for _ in range(REPEAT) for bb in range(BPC)]:
                # ---- q = W'.T-matmul on y[:, S-1] + pb[:, S-1] ----
                # tiny strided DMA: 512 scattered f32 (2 KB total)
                ylast = small.tile([128, DC, 1], fp32, tag="ylast")
                nc.sync.dma_start(
                    out=ylast,
                    in_=y[b, :, S - 1 : S].rearrange("(dc p) one -> p dc one", p=128),
                )
                q_ps = ps_small.tile([128, DC], fp32, tag="qps")
                for hc in range(DC):
                    for dc in range(DC):
                        nc.tensor.matmul(
                            q_ps[:, hc : hc + 1],
                            lhsT=wpt_sb[:, dc, hc * 128 : (hc + 1) * 128],
                            rhs=ylast[:, dc, :],
                            start=(dc == 0),
                            stop=(dc == DC - 1),
                        )
                q_sb = small.tile([128, DC], fp32, tag="qsb")
                for hc in range(DC):
                    nc.vector.tensor_add(
                        out=q_sb[:, hc : hc + 1],
                        in0=q_ps[:, hc : hc + 1],
                        in1=pb_sb[:, hc, S - 1 : S],
                    )

                # ---- v = W'.T @ q ----
                v_ps = ps_small.tile([128, DC], fp32, tag="vps")
                for dc in range(DC):
                    for hc in range(DC):
                        nc.tensor.matmul(
                            v_ps[:, dc : dc + 1],
                            lhsT=wp_sb[:, hc, dc * 128 : (dc + 1) * 128],
                            rhs=q_sb[:, hc : hc + 1],
                            start=(hc == 0),
                            stop=(hc == DC - 1),
                        )
                v_sb = small.tile([128, DC], fp32, tag="vsb")
                nc.vector.tensor_copy(out=v_sb, in_=v_ps)

                # ---- scores + softmax, tile by tile ----
                erow = rows.tile([1, S], fp32, tag="erow")
                asum = small.tile([1, ST], fp32, tag="asum")
                for st in range(ST):
                    # one 1-MB DMA per score tile: y[:, st*512:(st+1)*512] as
                    # [p, dc, s] (2-KB contiguous runs, 16-KB row stride)
                    yt = ypool.tile([128, DC, 512], fp32, tag="yt")
                    nc.sync.dma_start(
                        out=yt,
                        in_=y[b, :, st * 512 : (st + 1) * 512].rearrange(
                            "(dc p) s -> p dc s", p=128
                        ),
                    )
                    s_ps = ps_s.tile([1, 512], fp32, tag="sps")
                    for dc in range(DC):
                        nc.tensor.matmul(
                            s_ps,
                            lhsT=v_sb[:, dc : dc + 1],
                            rhs=yt[:, dc, :],
                            start=(dc == 0),
                            stop=False,
                        )
                    for hc in range(DC):
                        nc.tensor.matmul(
                            s_ps,
                            lhsT=q_sb[:, hc : hc + 1],
                            rhs=pb_sb[:, hc, st * 512 : (st + 1) * 512],
                            start=False,
                            stop=(hc == DC - 1),
                        )
                    # exp(scores/sqrt(H)) straight out of PSUM; free-dim sum
                    # accumulated into asum[:, st]. No max-subtraction: scores
                    # peak ~70 -> exp < 1.3e31, safely inside fp32 range.
                    nc.scalar.activation(
                        out=erow[:, st * 512 : (st + 1) * 512],
                        in_=s_ps,
                        func=mybir.ActivationFunctionType.Exp,
                        scale=INV_SQRT_H,
                        accum_out=asum[:, st : st + 1],
                    )

                tot = small.tile([1, 1], fp32, tag="tot")
                nc.vector.reduce_sum(out=tot, in_=asum, axis=mybir.AxisListType.X)
                rec = small.tile([1, 1], fp32, tag="rec")
                nc.vector.reciprocal(out=rec, in_=tot)
                nc.vector.tensor_scalar_mul(out=erow, in0=erow, scalar1=rec)
                nc.sync.dma_start(out=out[b : b + 1, :], in_=erow)

    _split_sync_waits(nc, mybir)
    return nc


def _get_program():
    global _CACHED
    if _CACHED is None:
        _CACHED = _build_program()
    return _CACHED


def kernel(t_current, t_history, y_current, y_history, Wk_w, Wk_b):
    global LAST_RESULT
    from concourse.bass_utils import run_bass_kernel_spmd

    y_history = np.asarray(y_history, dtype=np.float32)
    Wk_w = np.asarray(Wk_w, dtype=np.float32)
    Wk_b = np.asarray(Wk_b, dtype=np.float32)

    wp = Wk_w + np.eye(D, dtype=np.float32)  # fold "+ yh" into the weight
    wpt = np.ascontiguousarray(wp.T)
    pe = _sinusoidal_pe(S, D)
    pb = np.ascontiguousarray(pe.T) + Wk_b[:, None].astype(np.float32)

    nc = _get_program()
    in_maps = []
    for c in range(NCORES):
        in_maps.append(
            {
                "y": np.ascontiguousarray(y_history[c * BPC : (c + 1) * BPC]),
                "wp": wp,
                "wpt": wpt,
                "pb": np.ascontiguousarray(pb),
            }
        )
    res = run_bass_kernel_spmd(
        nc, in_maps, core_ids=list(range(NCORES)), trace=TRACE
    )
    LAST_RESULT = res
    return np.concatenate([r["out"] for r in res.results], axis=0)


# revision 15
# speedup vs baseline: 1.8521x; 1.8521x over previous
"""Trainium2 Bass kernel for nn_AttentionKernel_Position_47502338294174.

Reference computation (B=32, D=H=512, S=4096):
    yh = y_history.transpose(0, 2, 1)                 # [B,S,D]
    k  = yh @ Wk_w.T + Wk_b + yh + pe                 # [B,S,H]
    q  = k[:, -1, :]
    out = softmax((k @ q) / sqrt(H))                  # [B,S]

Key algebraic reduction (K is never materialized):
    W' = Wk_w + I                  (folds the "+ yh" term; H == D)
    pb = pe.T + Wk_b[:, None]      # [H, S] host constant
    q[h]      = sum_d W'[h,d] y[d,S-1] + pb[h,S-1]
    v         = W'.T @ q
    scores[s] = v . y[:, s] + q . pb[:, s]
    out       = softmax(scores / sqrt(H))

This turns a 68-GFLOP batched matmul into a matvec streamed over y_history
(268 MB) -> the kernel is HBM-bound at ~93us/core across 8 cores.

Sharding: pure data parallel, 4 batch elements per core; W'/pb replicated.
"""

import math

import numpy as np

B, D, S, H = 32, 512, 4096, 512
NCORES = 8
BPC = B // NCORES  # batches per core
INV_SQRT_H = 1.0 / math.sqrt(H)
DC = D // 128  # 4 contraction chunks
ST = S // 512  # 8 score tiles

# test.py can flip these before calling kernel()
TRACE = False
LAST_RESULT = None
REPEAT = 1  # perf harness: repeat the whole per-core workload in one NEFF

_CACHED = None  # (nc_program, ) built once per process


def _sinusoidal_pe(seq_len, d_model):
    pos = np.arange(seq_len, dtype=np.float32)[:, None]
    div = np.exp(
        np.arange(0, d_model, 2, dtype=np.float32) * (-math.log(10000.0) / d_model)
    ).astype(np.float32)
    pe = np.zeros((seq_len, d_model), dtype=np.float32)
    pe[:, 0::2] = np.sin(pos * div)
    pe[:, 1::2] = np.cos(pos * div)
    return pe


def _split_sync_waits(nc, mybir, max_waits=1):
    """The walrus build in this env rejects instructions carrying more than
    one sync-wait command. Hoist excess waits onto preceding same-engine NoOp
    carriers (sequential waits AND together -> identical semantics)."""
    n = 0
    for f in nc.m.functions:
        for blk in f.blocks:
            out = []
            for inst in blk.instructions:
                si = getattr(inst, "sync_info", None)
                if si is not None and si.on_wait and len(si.on_wait) > max_waits:
                    waits = list(si.on_wait)
                    while len(waits) > max_waits:
                        chunk, waits = waits[:max_waits], waits[max_waits:]
                        out.append(
                            mybir.InstNoOp(
                                name=f"{inst.name}-wsplit{n}",
                                engine=inst.engine,
                                ins=[],
                                outs=[],
                                sync_info=mybir.SyncInfo(
                                    on_wait=chunk, on_update=[]
                                ),
                            )
                        )
                        n += 1
                    si.on_wait = waits
                out.append(inst)
            blk.instructions = out
    return n


def _build_program():
    import concourse.bass as bass  # noqa: F401
    import concourse.mybir as mybir
    import concourse.tile as tile

    fp32 = mybir.dt.float32
    nc = bass.Bass(
        "TRN2",
        target_bir_lowering=False,
        debug=False,
        enable_asserts=False,
        num_devices=1,
    )

    y = nc.dram_tensor("y", (BPC, D, S), fp32, kind="ExternalInput").ap()
    wp = nc.dram_tensor("wp", (H, D), fp32, kind="ExternalInput").ap()
    wpt = nc.dram_tensor("wpt", (D, H), fp32, kind="ExternalInput").ap()
    pb = nc.dram_tensor("pb", (H, S), fp32, kind="ExternalInput").ap()
    out = nc.dram_tensor("out", (BPC, S), fp32, kind="ExternalOutput").ap()

    QS = S // 4  # quarter of the sequence, 1024

    with tile.TileContext(nc) as tc:
        with (
            tc.tile_pool(name="singles", bufs=1) as singles,
            tc.tile_pool(name="ypool", bufs=4) as ypool,
            tc.tile_pool(name="small", bufs=2) as small,
            tc.tile_pool(name="rows", bufs=1) as rows,
            tc.tile_pool(name="ps_qv", bufs=1, space="PSUM") as ps_qv,
            tc.tile_pool(name="ps_c", bufs=1, space="PSUM") as ps_c,
            tc.tile_pool(name="ps_s", bufs=2, space="PSUM") as ps_s,
        ):
            # ---- replicated constants (loaded once) ----
            wp_sb = singles.tile([128, DC, D], fp32)     # [p, hc, d] = W'[hc*128+p, d]
            nc.sync.dma_start(out=wp_sb, in_=wp.rearrange("(hc p) d -> p hc d", p=128))
            wpt_sb = singles.tile([128, DC, H], fp32)    # [p, dc, h] = W'.T[dc*128+p, h]
            nc.sync.dma_start(out=wpt_sb, in_=wpt.rearrange("(dc p) h -> p dc h", p=128))
            pb_sb = singles.tile([128, DC, S], fp32)     # [p, hc, s] = pb[hc*128+p, s]
            nc.sync.dma_start(out=pb_sb, in_=pb.rearrange("(hc p) s -> p hc s", p=128))

            for rep in range(REPEAT):
                # ---- q for all batches: [128, hc, b] = sum_dc W'T-chunk @ ylast ----
                ylast = small.tile([128, BPC, DC], fp32, tag="ylast")
                nc.sync.dma_start(
                    out=ylast,
                    in_=y[:, :, S - 1].rearrange("b (dc p) -> p b dc", p=128),
                )
                q_ps = ps_qv.tile([128, DC, BPC], fp32, tag="qps")
                for hc in range(DC):
                    for dc in range(DC):
                        nc.tensor.matmul(
                            q_ps[:, hc, :],
                            lhsT=wpt_sb[:, dc, hc * 128 : (hc + 1) * 128],
                            rhs=ylast[:, :, dc],
                            start=(dc == 0),
                            stop=(dc == DC - 1),
                        )
                q_sb = small.tile([128, DC, BPC], fp32, tag="qsb")
                for hc in range(DC):
                    # q += pb[:, S-1] (same value for every batch column)
                    nc.vector.tensor_scalar_add(
                        out=q_sb[:, hc, :],
                        in0=q_ps[:, hc, :],
                        scalar1=pb_sb[:, hc, S - 1 : S],
                    )

                # ---- v for all batches: [128, dc, b] = sum_hc W'-chunk @ q ----
                v_ps = ps_qv.tile([128, DC, BPC], fp32, tag="vps")
                for dc in range(DC):
                    for hc in range(DC):
                        nc.tensor.matmul(
                            v_ps[:, dc, :],
                            lhsT=wp_sb[:, hc, dc * 128 : (dc + 1) * 128],
                            rhs=q_sb[:, hc, :],
                            start=(hc == 0),
                            stop=(hc == DC - 1),
                        )
                v_sb = small.tile([128, DC, BPC], fp32, tag="vsb")
                nc.vector.tensor_copy(out=v_sb, in_=v_ps)

                # ---- scores + softmax ----
                # batch b lives on partition 32*b (engine ops must be
                # 32-aligned in partition; tile_position routes matmul
                # output to col-group 32*b).
                erow = rows.tile([128, S], fp32, tag="erow")
                asum = small.tile([128, 4], fp32, tag="asum")
                for h in range(4):
                    sl = slice(h * QS, (h + 1) * QS)
                    # shared across batches: c[i, s] = q_i . pb[:, s]
                    c_ps = ps_c.tile([BPC, QS], fp32, tag="cps")
                    for j in range(QS // 512):
                        for hc in range(DC):
                            nc.tensor.matmul(
                                c_ps[:, j * 512 : (j + 1) * 512],
                                lhsT=q_sb[:, hc, :],
                                rhs=pb_sb[:, hc, h * QS + j * 512 : h * QS + (j + 1) * 512],
                                start=(hc == 0),
                                stop=(hc == DC - 1),
                            )
                    c_sb = small.tile([BPC, QS], fp32, tag="csb")
                    nc.vector.tensor_copy(out=c_sb, in_=c_ps)
                    # spread batch rows 0..3 -> partitions 0,32,64,96
                    c_sp = small.tile([128, QS], fp32, tag="csp")
                    for b in range(BPC):
                        nc.sync.dma_start(
                            out=c_sp[32 * b : 32 * b + 1, :], in_=c_sb[b : b + 1, :]
                        )
                    s_ps = ps_s.tile([128, QS], fp32, tag="sps")
                    for b in range(BPC):
                        # y quarter-slab for this batch: 2-MB DMA, 4-KB runs
                        yt = ypool.tile([128, DC, QS], fp32, tag="yt")
                        nc.sync.dma_start(
                            out=yt,
                            in_=y[b, :, sl].rearrange("(dc p) s -> p dc s", p=128),
                        )
                        for j in range(QS // 512):
                            for dc in range(DC):
                                nc.tensor.matmul(
                                    s_ps[32 * b : 32 * b + 1, j * 512 : (j + 1) * 512],
                                    lhsT=v_sb[:, dc, b : b + 1],
                                    rhs=yt[:, dc, j * 512 : (j + 1) * 512],
                                    start=(dc == 0),
                                    stop=(dc == DC - 1),
                                    tile_position=(0, 32 * b),
                                )
                        nc.vector.tensor_add(
                            out=s_ps[32 * b : 32 * b + 1, :],
                            in0=s_ps[32 * b : 32 * b + 1, :],
                            in1=c_sp[32 * b : 32 * b + 1, :],
                        )
                        # exp(scores/sqrt(H)); fused free-dim sum into asum.
                        # No max-subtraction: scores peak ~70 -> exp < 1.3e31,
                        # safely inside fp32 range.
                        nc.scalar.activation(
                            out=erow[32 * b : 32 * b + 1, sl],
                            in_=s_ps[32 * b : 32 * b + 1, :],
                            func=mybir.ActivationFunctionType.Exp,
                            scale=INV_SQRT_H,
                            accum_out=asum[32 * b : 32 * b + 1, h : h + 1],
                        )

                tot = small.tile([128, 1], fp32, tag="tot")
                nc.vector.reduce_sum(out=tot, in_=asum, axis=mybir.AxisListType.X)
                rec = small.tile([128, 1], fp32, tag="rec")
                nc.vector.reciprocal(out=rec, in_=tot)
                nc.vector.tensor_scalar_mul(out=erow, in0=erow, scalar1=rec)
                for b in range(BPC):
                    nc.sync.dma_start(
                        out=out[b : b + 1, :], in_=erow[32 * b : 32 * b + 1, :]
                    )

    _split_sync_waits(nc, mybir)
    return nc


def _get_program():
    global _CACHED
    if _CACHED is None:
        _CACHED = _build_program()
    return _CACHED


def kernel(t_current, t_history, y_current, y_history, Wk_w, Wk_b):
    global LAST_RESULT
    from concourse.bass_utils import run_bass_kernel_spmd

    y_history = np.asarray(y_history, dtype=np.float32)
    Wk_w = np.asarray(Wk_w, dtype=np.float32)
    Wk_b = np.asarray(Wk_b, dtype=np.float32)

    wp = Wk_w + np.eye(D, dtype=np.float32)  # fold "+ yh" into the weight
    wpt = np.ascontiguousarray(wp.T)
    pe = _sinusoidal_pe(S, D)
    pb = np.ascontiguousarray(pe.T) + Wk_b[:, None].astype(np.float32)

    nc = _get_program()
    in_maps = []
    for c in range(NCORES):
        in_maps.append(
            {
                "y": np.ascontiguousarray(y_history[c * BPC : (c + 1) * BPC]),
                "wp": wp,
                "wpt": wpt,
                "pb": np.ascontiguousarray(pb),
            }
        )
    res = run_bass_kernel_spmd(
        nc, in_maps, core_ids=list(range(NCORES)), trace=TRACE
    )
    LAST_RESULT = res
    return np.concatenate([r["out"] for r in res.results], axis=0)


# revision 17
# speedup vs baseline: 3.6025x; 1.9451x over previous
"""Trainium2 Bass kernel for nn_AttentionKernel_Position_47502338294174.

Reference computation (B=32, D=H=512, S=4096):
    yh = y_history.transpose(0, 2, 1)                 # [B,S,D]
    k  = yh @ Wk_w.T + Wk_b + yh + pe                 # [B,S,H]
    q  = k[:, -1, :]
    out = softmax((k @ q) / sqrt(H))                  # [B,S]

Key algebraic reduction (K is never materialized):
    W' = Wk_w + I                  (folds the "+ yh" term; H == D)
    pb = pe.T + Wk_b[:, None]      # [H, S] host constant
    q[h]      = sum_d W'[h,d] y[d,S-1] + pb[h,S-1]
    v         = W'.T @ q
    scores[s] = v . y[:, s] + q . pb[:, s]
    out       = softmax(scores / sqrt(H))

This turns a 68-GFLOP batched matmul into a matvec streamed over y_history
(268 MB) -> the kernel is HBM-bound at ~93us/core across 8 cores.

Sharding: pure data parallel, 4 batch elements per core; W'/pb replicated.
"""

import math

import numpy as np

B, D, S, H = 32, 512, 4096, 512
NCORES = 8
BPC = B // NCORES  # batches per core
INV_SQRT_H = 1.0 / math.sqrt(H)
DC = D // 128  # 4 contraction chunks
ST = S // 512  # 8 score tiles

# test.py can flip these before calling kernel()
TRACE = False
LAST_RESULT = None
REPEAT = 1  # perf harness: repeat the whole per-core workload in one NEFF

_CACHED = None  # (nc_program, ) built once per process


def _sinusoidal_pe(seq_len, d_model):
    pos = np.arange(seq_len, dtype=np.float32)[:, None]
    div = np.exp(
        np.arange(0, d_model, 2, dtype=np.float32) * (-math.log(10000.0) / d_model)
    ).astype(np.float32)
    pe = np.zeros((seq_len, d_model), dtype=np.float32)
    pe[:, 0::2] = np.sin(pos * div)
    pe[:, 1::2] = np.cos(pos * div)
    return pe


def _split_sync_waits(nc, mybir, max_waits=1):
    """The walrus build in this env rejects instructions carrying more than
    one sync-wait command. Hoist excess waits onto preceding same-engine NoOp
    carriers (sequential waits AND together -> identical semantics)."""
    n = 0
    for f in nc.m.functions:
        for blk in f.blocks:
            out = []
            for inst in blk.instructions:
                si = getattr(inst, "sync_info", None)
                if si is not None and si.on_wait and len(si.on_wait) > max_waits:
                    waits = list(si.on_wait)
                    while len(waits) > max_waits:
                        chunk, waits = waits[:max_waits], waits[max_waits:]
                        out.append(
                            mybir.InstNoOp(
                                name=f"{inst.name}-wsplit{n}",
                                engine=inst.engine,
                                ins=[],
                                outs=[],
                                sync_info=mybir.SyncInfo(
                                    on_wait=chunk, on_update=[]
                                ),
                            )
                        )
                        n += 1
                    si.on_wait = waits
                out.append(inst)
            blk.instructions = out
    return n


def _build_program():
    import concourse.bass as bass  # noqa: F401
    import concourse.mybir as mybir
    import concourse.tile as tile

    fp32 = mybir.dt.float32
    nc = bass.Bass(
        "TRN2",
        target_bir_lowering=False,
        debug=False,
        enable_asserts=False,
        num_devices=1,
    )

    y = nc.dram_tensor("y", (BPC, D, S), fp32, kind="ExternalInput").ap()
    wp = nc.dram_tensor("wp", (H, D), fp32, kind="ExternalInput").ap()
    wpt = nc.dram_tensor("wpt", (D, H), fp32, kind="ExternalInput").ap()
    pb = nc.dram_tensor("pb", (H, S), fp32, kind="ExternalInput").ap()
    out = nc.dram_tensor("out", (BPC, S), fp32, kind="ExternalOutput").ap()

    HS = S // 2  # half row, 2048

    with tile.TileContext(nc) as tc:
        with (
            tc.tile_pool(name="singles", bufs=1) as singles,
            tc.tile_pool(name="ypool", bufs=4) as ypool,
            tc.tile_pool(name="small", bufs=2) as small,
            tc.tile_pool(name="rows", bufs=1) as rows,
            tc.tile_pool(name="ps_qv", bufs=1, space="PSUM") as ps_qv,
            tc.tile_pool(name="ps_c", bufs=2, space="PSUM") as ps_c,
            tc.tile_pool(name="ps_s", bufs=1, space="PSUM") as ps_s,
        ):
            # ---- replicated constants (loaded once) ----
            wp_sb = singles.tile([128, DC, D], fp32)     # [p, hc, d] = W'[hc*128+p, d]
            nc.sync.dma_start(out=wp_sb, in_=wp.rearrange("(hc p) d -> p hc d", p=128))
            wpt_sb = singles.tile([128, DC, H], fp32)    # [p, dc, h] = W'.T[dc*128+p, h]
            nc.sync.dma_start(out=wpt_sb, in_=wpt.rearrange("(dc p) h -> p dc h", p=128))
            pb_sb = singles.tile([128, DC, S], fp32)     # [p, hc, s] = pb[hc*128+p, s]
            nc.sync.dma_start(out=pb_sb, in_=pb.rearrange("(hc p) s -> p hc s", p=128))

            for rep in range(REPEAT):
                # ---- q for all batches: [128, hc, b] = sum_dc W'T-chunk @ ylast ----
                ylast = small.tile([128, BPC, DC], fp32, tag="ylast")
                nc.sync.dma_start(
                    out=ylast,
                    in_=y[:, :, S - 1].rearrange("b (dc p) -> p b dc", p=128),
                )
                q_ps = ps_qv.tile([128, DC, BPC], fp32, tag="qps")
                for hc in range(DC):
                    for dc in range(DC):
                        nc.tensor.matmul(
                            q_ps[:, hc, :],
                            lhsT=wpt_sb[:, dc, hc * 128 : (hc + 1) * 128],
                            rhs=ylast[:, :, dc],
                            start=(dc == 0),
                            stop=(dc == DC - 1),
                        )
                q_sb = small.tile([128, DC, BPC], fp32, tag="qsb")
                for hc in range(DC):
                    # q += pb[:, S-1] (same value for every batch column)
                    nc.vector.tensor_scalar_add(
                        out=q_sb[:, hc, :],
                        in0=q_ps[:, hc, :],
                        scalar1=pb_sb[:, hc, S - 1 : S],
                    )

                # ---- v for all batches: [128, dc, b] = sum_hc W'-chunk @ q ----
                v_ps = ps_qv.tile([128, DC, BPC], fp32, tag="vps")
                for dc in range(DC):
                    for hc in range(DC):
                        nc.tensor.matmul(
                            v_ps[:, dc, :],
                            lhsT=wp_sb[:, hc, dc * 128 : (dc + 1) * 128],
                            rhs=q_sb[:, hc, :],
                            start=(hc == 0),
                            stop=(hc == DC - 1),
                        )
                v_sb = small.tile([128, DC, BPC], fp32, tag="vsb")
                nc.vector.tensor_copy(out=v_sb, in_=v_ps)

                # ---- shared pebias term, full row: c[i, s] = q_i . pb[:, s] ----
                # computed once for all batches, spread to partitions 0/32/64/96
                c_sb = rows.tile([BPC, S], fp32, tag="csb")
                for st in range(S // 512):
                    c_ps = ps_c.tile([BPC, 512], fp32, tag="cps")
                    for hc in range(DC):
                        nc.tensor.matmul(
                            c_ps,
                            lhsT=q_sb[:, hc, :],
                            rhs=pb_sb[:, hc, st * 512 : (st + 1) * 512],
                            start=(hc == 0),
                            stop=(hc == DC - 1),
                        )
                    nc.vector.tensor_copy(
                        out=c_sb[:, st * 512 : (st + 1) * 512], in_=c_ps
                    )
                c_sp = rows.tile([128, S], fp32, tag="csp")
                for b in range(BPC):
                    nc.sync.dma_start(
                        out=c_sp[32 * b : 32 * b + 1, :], in_=c_sb[b : b + 1, :]
                    )

                # ---- scores + softmax; batch b lives on partition 32*b ----
                erow = rows.tile([128, S], fp32, tag="erow")
                asum = small.tile([128, 2], fp32, tag="asum")
                for b in range(BPC):
                    # linear 2-MB DMAs: one per 128-row d-chunk
                    ytiles = []
                    for dc in range(DC):
                        yt = ypool.tile([128, S], fp32, tag="yt")
                        nc.sync.dma_start(
                            out=yt, in_=y[b, dc * 128 : (dc + 1) * 128, :]
                        )
                        ytiles.append(yt)
                    for h in range(2):
                        sl = slice(h * HS, (h + 1) * HS)
                        s_ps = ps_s.tile([128, HS], fp32, tag="sps")
                        for j in range(HS // 512):
                            for dc in range(DC):
                                nc.tensor.matmul(
                                    s_ps[32 * b : 32 * b + 1, j * 512 : (j + 1) * 512],
                                    lhsT=v_sb[:, dc, b : b + 1],
                                    rhs=ytiles[dc][
                                        :, h * HS + j * 512 : h * HS + (j + 1) * 512
                                    ],
                                    start=(dc == 0),
                                    stop=(dc == DC - 1),
                                    tile_position=(0, 32 * b),
                                )
                        nc.vector.tensor_add(
                            out=s_ps[32 * b : 32 * b + 1, :],
                            in0=s_ps[32 * b : 32 * b + 1, :],
                            in1=c_sp[32 * b : 32 * b + 1, sl],
                        )
                        # exp(scores/sqrt(H)); fused free-dim sum into asum.
                        # No max-subtraction: scores peak ~70 -> exp < 1.3e31,
                        # safely inside fp32 range.
                        nc.scalar.activation(
                            out=erow[32 * b : 32 * b + 1, sl],
                            in_=s_ps[32 * b : 32 * b + 1, :],
                            func=mybir.ActivationFunctionType.Exp,
                            scale=INV_SQRT_H,
                            accum_out=asum[32 * b : 32 * b + 1, h : h + 1],
                        )

                tot = small.tile([128, 1], fp32, tag="tot")
                nc.vector.reduce_sum(out=tot, in_=asum, axis=mybir.AxisListType.X)
                rec = small.tile([128, 1], fp32, tag="rec")
                nc.vector.reciprocal(out=rec, in_=tot)
                nc.vector.tensor_scalar_mul(out=erow, in0=erow, scalar1=rec)
                for b in range(BPC):
                    nc.sync.dma_start(
                        out=out[b : b + 1, :], in_=erow[32 * b : 32 * b + 1, :]
                    )

    _split_sync_waits(nc, mybir)
    return nc


def _get_program():
    global _CACHED
    if _CACHED is None:
        _CACHED = _build_program()
    return _CACHED


def kernel(t_current, t_history, y_current, y_history, Wk_w, Wk_b):
    global LAST_RESULT
    from concourse.bass_utils import run_bass_kernel_spmd

    y_history = np.asarray(y_history, dtype=np.float32)
    Wk_w = np.asarray(Wk_w, dtype=np.float32)
    Wk_b = np.asarray(Wk_b, dtype=np.float32)

    wp = Wk_w + np.eye(D, dtype=np.float32)  # fold "+ yh" into the weight
    wpt = np.ascontiguousarray(wp.T)
    pe = _sinusoidal_pe(S, D)
    pb = np.ascontiguousarray(pe.T) + Wk_b[:, None].astype(np.float32)

    nc = _get_program()
    in_maps = []
    for c in range(NCORES):
        in_maps.append(
            {
                "y": np.ascontiguousarray(y_history[c * BPC : (c + 1) * BPC]),
                "wp": wp,
                "wpt": wpt,
                "pb": np.ascontiguousarray(pb),
            }
        )
    res = run_bass_kernel_spmd(
        nc, in_maps, core_ids=list(range(NCORES)), trace=TRACE
    )
    LAST_RESULT = res
    return np.concatenate([r["out"] for r in res.results], axis=0)
